# revision 44
# baseline (speedup 1.0000x reference)
"""Trainium2 Bass kernel for nn_ClusteringLayer (vq_codebook, Student-t assignments).

Computes, for x in R^{N x D} and clusters c in R^{K x D}:
    d2[n,k] = ||x_n - c_k||^2
    q = 1 / (1 + d2)            (Student-t, alpha=1, power=(alpha+1)/2=1)
    out = q / q.sum(-1, keepdims=True)

Strategy (data-parallel over 8 NeuronCores, cluster table replicated):
  - host: shard x along N (65536 rows/core), transpose+cast each shard to
    f16 X^T [D, Nsh]; precompute x2[n] = ||x_n||^2 (f16) and the cluster
    constants (-2C)^T (f16 stationary) and 1+||c_k||^2.
  - device, per 2048-column block, with psum packed [2 groups x 64 k]:
      psum[kp, j] = sum_d (-2c)[d,k] x[d,n]  +  (1 + c2[k]) + x2[n]
                    (2 main matmuls + 1 tiny K=2 "aug" matmul per 512-slice;
                     the aug matmul streams [ones; x2] against [c2p1; ones])
      q  = 1/psum                    (DVE reciprocal, f16)          [n-major: no!]
      s  = indicator-matmul over the k-partitions -> [2, 512] psums
      sinv = 1/s                     (DVE)
      bcast = selT-matmul            (replicates sinv to all 64 k rows)
      out = q * bcast                (DVE)  -> accumulated in slab tile
  - slab-grained (8192 cols) input/output DMAs for large descriptors.
  - host: upcast f16 -> f32, unpack [k, n] blocks to [n, k], concat shards.
"""

import numpy as np
from contextlib import ExitStack


def _patch_act_tables():
    """Make Ln and Exp resolve to the single set that contains both
    (natural_log_exp_and_others), so the kernel pays one ACT_TABLE_LOAD
    instead of alternating sets every block.  Only values are modified --
    set order (and hence act_func_set_id indices) is preserved."""
    import functools
    from concourse import hw_specs, bacc, mybir

    if getattr(hw_specs, "_act_tables_patched", False):
        return
    orig = hw_specs.get_activation_tables

    @functools.cache
    def patched(arch):
        t = dict(orig(arch))
        ln = mybir.ActivationFunctionType.Ln
        ex = mybir.ActivationFunctionType.Exp
        out = {}
        for name, funcs in t.items():
            if name != "natural_log_exp_and_others" and (ln in funcs or ex in funcs):
                funcs = funcs - {ln, ex}
            out[name] = funcs
        return out

    hw_specs.get_activation_tables = patched
    bacc.get_activation_tables = patched
    hw_specs._act_tables_patched = True

N, D, K = 524288, 256, 64
NCORES = 8
NSH = N // NCORES      # 65536 rows per core
BLK = 2048             # n-columns per psum block (2 groups x 1024)
SLAB = 8192            # n-columns per DMA slab (4 blocks)
NBLK = NSH // BLK      # 32
NSLAB = NSH // SLAB    # 8
BPS = SLAB // BLK      # blocks per slab = 4


def _build(nsh=NSH, blk=BLK, slab=SLAB):
    import concourse.bacc as bacc
    import concourse.tile as tile
    from concourse import mybir

    _patch_act_tables()

    f32 = mybir.dt.float32
    f16 = mybir.dt.float16
    xdt = mybir.dt.bfloat16   # dtype of the main GEMM path (x, ct, aug, x2)
    nblk = nsh // blk
    bps = slab // blk
    nslab = nsh // slab
    half = blk // 2            # 1024 = columns per psum group
    qcols = half               # q/psum free size
    outw = bps * half          # out-slab width (1024 per block)

    nc = bacc.Bacc("TRN2", target_bir_lowering=False, debug=False)
    # aug/selt are zero-padded to K=128: matmuls with tiny contraction (K=2)
    # keep the PE activity monitor from ever un-throttling the clock (HW-
    # measured: a 1/3 mix of K=2 MMs pins every MM at 1.2 GHz instead of 2.4).
    xt = nc.dram_tensor("xt", [D, nsh], xdt, kind="ExternalInput").ap()
    ctb = nc.dram_tensor("ctb", [128, 128], xdt, kind="ExternalInput").ap()
    aug = nc.dram_tensor("aug", [128, K], xdt, kind="ExternalInput").ap()
    x2d = nc.dram_tensor("x2d", [2, nsh], xdt, kind="ExternalInput").ap()
    sel = nc.dram_tensor("sel", [128, 32], f16, kind="ExternalInput").ap()
    selt = nc.dram_tensor("selt", [128, 128], f16, kind="ExternalInput").ap()
    selt2 = nc.dram_tensor("selt2", [128, 128], f16, kind="ExternalInput").ap()
    qo = nc.dram_tensor("q", [128, nsh // 2], f16, kind="ExternalOutput").ap()

    with tile.TileContext(nc) as tc, ExitStack() as ctx, \
            nc.allow_low_precision(reason="f16 q validated against reference"):
        wp = ctx.enter_context(tc.tile_pool(name="w", bufs=1))
        io = ctx.enter_context(tc.tile_pool(name="io", bufs=3))
        qp = ctx.enter_context(tc.tile_pool(name="qp", bufs=4))
        sv = ctx.enter_context(tc.tile_pool(name="sv", bufs=2))
        op = ctx.enter_context(tc.tile_pool(name="op", bufs=2))
        pp = ctx.enter_context(tc.tile_pool(name="ps", bufs=2, space="PSUM"))
        sp = ctx.enter_context(tc.tile_pool(name="sp", bufs=1, space="PSUM"))
        bp = ctx.enter_context(tc.tile_pool(name="bp", bufs=1, space="PSUM"))

        # one-time constants
        ctb_sb = wp.tile([128, 128], xdt, tag="ctb")
        nc.sync.dma_start(ctb_sb, ctb)
        aug_sb = wp.tile([128, K], xdt, tag="aug")
        nc.sync.dma_start(aug_sb, aug)
        sel_sb = wp.tile([128, 32], f16, tag="sel")
        nc.sync.dma_start(sel_sb, sel)
        selt_sb = wp.tile([128, 128], f16, tag="selt")
        nc.sync.dma_start(selt_sb, selt)
        selt2_sb = wp.tile([128, 128], f16, tag="selt2")
        nc.sync.dma_start(selt2_sb, selt2)

        # Manually double-buffered K=128 moving tiles whose rows 2:128 are
        # multiplied by stationary zeros: memset once so the garbage rows
        # can never be NaN (0 * NaN = NaN would poison the psum).
        x2ts = [wp.tile([128, slab], xdt, tag=f"x2t{j}", name=f"x2t{j}")
                for j in range(2)]
        nc.vector.memset(x2ts[0], 0.0)
        nc.gpsimd.memset(x2ts[1], 0.0)
        sis = [wp.tile([128, 512], f16, tag=f"si{j}", name=f"si{j}")
               for j in range(2)]
        for j in range(2):
            nc.vector.memset(sis[j], 0.0)

        xs = [None] * nslab    # (xt0s, xt1s) per live slab
        x2s = [None] * nslab   # x2 tile per live slab
        qs = [None] * nblk     # q tile per live block
        svs = [None] * nblk    # (sinv_h0, sinv_h1)
        outs = [None] * nslab  # out slab tile

        def load_slab(s):
            x0 = io.tile([128, slab], xdt, tag="xt0")
            x1 = io.tile([128, slab], xdt, tag="xt1")
            nc.sync.dma_start(x0, xt[0:128, s * slab:(s + 1) * slab])
            nc.sync.dma_start(x1, xt[128:256, s * slab:(s + 1) * slab])
            xs[s] = (x0, x1)

        def load_x2(s):
            # x2ts are manually double-buffered: this DMA must be emitted
            # only after every aug-MM read of slab s-2 (same buffer) is
            # already in the program, or the scheduler will order those
            # reads AFTER this write (reading slab s's x2 for slab s-2).
            x2t = x2ts[s % 2]
            nc.sync.dma_start(x2t[0:2, :], x2d[:, s * slab:(s + 1) * slab])
            x2s[s] = x2t

        def main_mms(b):
            s, bi = b // bps, b % bps
            x0, x1 = xs[s]
            x2t = x2s[s]
            ps = pp.tile([128, half], f32, tag="d2")
            for g in range(2):
                rows = slice(g * K, (g + 1) * K)
                tpos = (0, g * K) if g else None
                for h in range(2):
                    sl = slice(h * 512, (h + 1) * 512)
                    xoff = bi * blk + g * half + h * 512
                    xsl = slice(xoff, xoff + 512)
                    nc.tensor.matmul(ps[rows, sl], ctb_sb[:, 0:K], x0[:, xsl],
                                     start=True, stop=False, tile_position=tpos)
                    nc.tensor.matmul(ps[rows, sl], ctb_sb[:, K:2 * K], x1[:, xsl],
                                     start=False, stop=False, tile_position=tpos)
                    nc.tensor.matmul(ps[rows, sl], aug_sb, x2t[:, xsl],
                                     start=False, stop=True, tile_position=tpos)
            # q = 1/(1+d2) = exp(-ln(1+d2)) on the (otherwise idle) ACT engine
            u = qp.tile([128, qcols], f16, tag="u")
            nc.scalar.activation(u, ps, func=mybir.ActivationFunctionType.Ln,
                                 scale=1.0)
            q = qp.tile([128, qcols], f16, tag="q")
            nc.scalar.activation(q, u, func=mybir.ActivationFunctionType.Exp,
                                 scale=-1.0)
            qs[b] = q

        def norm_a(b):
            # both 512-halves' group sums packed into one [34, 512] psum:
            # h0 at rows 0:2, h1 at rows 32:34 (base-partition-32 aligned),
            # so one approx + one cast cover both.
            # sel's columns 2:32 are all-ones dummies so rows 2:32 of the
            # psum hold finite sums -- the packed reciprocal below must not
            # see stale/zero psum (1/0 -> inf -> 0*inf = NaN at the bcast).
            q = qs[b]
            st = sp.tile([64, 512], f32, tag="sh")
            nc.tensor.matmul(st[0:32, :], sel_sb, q[:, 0:512],
                             start=True, stop=True)
            nc.tensor.matmul(st[32:64, :], sel_sb, q[:, 512:1024],
                             start=True, stop=True, tile_position=(0, 32))
            sf = sv.tile([34, 512], f32, tag="sf")
            nc.vector.reciprocal_approx_fast(sf, st[0:34, :])
            si = sis[b % 2]
            nc.vector.tensor_copy(si[0:34, :], sf)
            svs[b] = si

        def norm_b(b):
            s, bi = b // bps, b % bps
            si = svs[b]
            bc = bp.tile([128, half], f32, tag="bc")
            nc.tensor.matmul(bc[:, 0:512], selt_sb, si, start=True, stop=True)
            nc.tensor.matmul(bc[:, 512:1024], selt2_sb, si,
                             start=True, stop=True)
            if bi == 0:
                ot = op.tile([128, outw], f16, tag="out")
                outs[s] = ot
            nc.vector.tensor_tensor(outs[s][:, bi * half:(bi + 1) * half],
                                    qs[b], bc, op=mybir.AluOpType.mult)
            if bi == bps - 1:
                nc.scalar.dma_start(
                    qo[:, s * outw:(s + 1) * outw], outs[s])

        # slab 0 loads in block-sized pieces so block 0's matmuls can start
        # after ~0.5 MB instead of 2 MB per half.
        x0f = io.tile([128, slab], xdt, tag="xt0")
        x1f = io.tile([128, slab], xdt, tag="xt1")
        for bi in range(bps):
            cs = slice(bi * blk, (bi + 1) * blk)
            nc.sync.dma_start(x0f[:, cs], xt[0:128, cs])
            nc.sync.dma_start(x1f[:, cs], xt[128:256, cs])
        xs[0] = (x0f, x1f)
        load_x2(0)
        if nslab > 1:
            load_slab(1)
            load_x2(1)
        if nslab > 2:
            load_slab(2)

        # PE warm-up burst: ~5us of dependency-free matmuls so HAM reaches
        # K=8/8 while the first input slab is still loading.
        wmw = wp.tile([128, K], xdt, tag="wmw")
        nc.vector.memset(wmw, 0.0)
        wmm = wp.tile([128, 512], xdt, tag="wmm")
        nc.vector.memset(wmm, 0.0)
        wps = pp.tile([128, half], f32, tag="d2")
        for j in range(24):
            nc.tensor.matmul(wps[0:K, 0:512], wmw, wmm,
                             start=True, stop=True)

        for i in range(nblk + 3):
            if i < nblk:
                if i % bps == 0 and i // bps + 3 < nslab:
                    load_slab(i // bps + 3)
                main_mms(i)
                if (i + 1) % bps == 0 and i // bps + 2 < nslab:
                    load_x2(i // bps + 2)
            if 2 <= i and i - 2 < nblk:
                norm_a(i - 2)
            if 3 <= i:
                norm_b(i - 3)

    nc.compile()
    return nc


_CACHE = {}


def _get_nc():
    if "nc" not in _CACHE:
        _CACHE["nc"] = _build()
    return _CACHE["nc"]


def _prep_inputs(x, c):
    """Build per-core input maps (host-side shard + layout prep)."""
    import ml_dtypes

    f16 = np.float16
    xdt = ml_dtypes.bfloat16
    x = np.asarray(x, dtype=np.float32)
    c = np.asarray(c, dtype=np.float32)
    assert x.shape == (N, D) and c.shape == (K, D)

    c2p1 = 1.0 + np.sum(c * c, axis=1)                     # (K,)
    aug = np.zeros((128, K), np.float32)                   # K=128 zero-padded
    aug[0] = c2p1
    aug[1] = 1.0
    aug = aug.astype(xdt)
    ctb = np.ascontiguousarray(
        np.concatenate([(-2.0 * c).T[0:128, :], (-2.0 * c).T[128:256, :]],
                       axis=1)).astype(xdt)                # [128, 128]
    sel = np.ones((128, 32), f16)      # cols 2:32 = dummy all-ones columns
    sel[:, 0] = 0.0
    sel[:, 1] = 0.0
    sel[0:K, 0] = 1.0
    sel[K:128, 1] = 1.0
    selt = np.zeros((128, 128), f16)                       # K=128 zero-padded
    selt[0, 0:K] = 1.0
    selt[1, K:128] = 1.0
    selt2 = np.zeros((128, 128), f16)                      # h1 variant (rows 32/33)
    selt2[32, 0:K] = 1.0
    selt2[33, K:128] = 1.0

    x2 = np.sum(x.astype(np.float32) ** 2, axis=1)         # (N,) f32

    in_maps = []
    for i in range(NCORES):
        xsh = x[i * NSH:(i + 1) * NSH]
        xts = np.ascontiguousarray(xsh.T.astype(xdt))      # [D, NSH]
        x2d = np.empty((2, NSH), xdt)
        x2d[0] = 1.0
        x2d[1] = x2[i * NSH:(i + 1) * NSH].astype(xdt)
        in_maps.append({"xt": xts, "ctb": ctb, "aug": aug, "x2d": x2d,
                        "sel": sel, "selt": selt, "selt2": selt2})
    return in_maps


def _postprocess(results):
    """[128, NSH/2] f16 per core -> full [N, K] f32."""
    outs = []
    for r in results:
        qt = np.asarray(r["q"]).astype(np.float32)          # [128, 32768]
        a = qt.reshape(2, K, NSLAB, BPS, BLK // 2)          # [g, k, s, bi, j]
        outs.append(a.transpose(2, 3, 0, 4, 1).reshape(NSH, K))
    return np.concatenate(outs, axis=0)


def kernel(inputs, clusters):
    from concourse.bass_utils import run_bass_kernel_spmd

    nc = _get_nc()
    in_maps = _prep_inputs(inputs, clusters)
    res = run_bass_kernel_spmd(nc, in_maps, core_ids=list(range(NCORES)))
    return _postprocess(res.results)


# revision 45
# speedup vs baseline: 1.0133x; 1.0133x over previous
"""Trainium2 Bass kernel for nn_ClusteringLayer (vq_codebook, Student-t assignments).

Computes, for x in R^{N x D} and clusters c in R^{K x D}:
    d2[n,k] = ||x_n - c_k||^2
    q = 1 / (1 + d2)            (Student-t, alpha=1, power=(alpha+1)/2=1)
    out = q / q.sum(-1, keepdims=True)

Strategy (data-parallel over 8 NeuronCores, cluster table replicated):
  - host: shard x along N (65536 rows/core), transpose+cast each shard to
    f16 X^T [D, Nsh]; precompute x2[n] = ||x_n||^2 (f16) and the cluster
    constants (-2C)^T (f16 stationary) and 1+||c_k||^2.
  - device, per 2048-column block, with psum packed [2 groups x 64 k]:
      psum[kp, j] = sum_d (-2c)[d,k] x[d,n]  +  (1 + c2[k]) + x2[n]
                    (2 main matmuls + 1 tiny K=2 "aug" matmul per 512-slice;
                     the aug matmul streams [ones; x2] against [c2p1; ones])
      q  = 1/psum                    (DVE reciprocal, f16)          [n-major: no!]
      s  = indicator-matmul over the k-partitions -> [2, 512] psums
      sinv = 1/s                     (DVE)
      bcast = selT-matmul            (replicates sinv to all 64 k rows)
      out = q * bcast                (DVE)  -> accumulated in slab tile
  - slab-grained (8192 cols) input/output DMAs for large descriptors.
  - host: upcast f16 -> f32, unpack [k, n] blocks to [n, k], concat shards.
"""

import numpy as np
from contextlib import ExitStack


def _patch_act_tables():
    """Make Ln and Exp resolve to the single set that contains both
    (natural_log_exp_and_others), so the kernel pays one ACT_TABLE_LOAD
    instead of alternating sets every block.  Only values are modified --
    set order (and hence act_func_set_id indices) is preserved."""
    import functools
    from concourse import hw_specs, bacc, mybir

    if getattr(hw_specs, "_act_tables_patched", False):
        return
    orig = hw_specs.get_activation_tables

    @functools.cache
    def patched(arch):
        t = dict(orig(arch))
        ln = mybir.ActivationFunctionType.Ln
        ex = mybir.ActivationFunctionType.Exp
        out = {}
        for name, funcs in t.items():
            if name != "natural_log_exp_and_others" and (ln in funcs or ex in funcs):
                funcs = funcs - {ln, ex}
            out[name] = funcs
        return out

    hw_specs.get_activation_tables = patched
    bacc.get_activation_tables = patched
    hw_specs._act_tables_patched = True

N, D, K = 524288, 256, 64
NCORES = 8
NSH = N // NCORES      # 65536 rows per core
BLK = 2048             # n-columns per psum block (2 groups x 1024)
SLAB = 8192            # n-columns per DMA slab (4 blocks)
NBLK = NSH // BLK      # 32
NSLAB = NSH // SLAB    # 8
BPS = SLAB // BLK      # blocks per slab = 4


def _build(nsh=NSH, blk=BLK, slab=SLAB):
    import concourse.bacc as bacc
    import concourse.tile as tile
    from concourse import mybir

    _patch_act_tables()

    f32 = mybir.dt.float32
    f16 = mybir.dt.float16
    xdt = mybir.dt.bfloat16   # dtype of the main GEMM path (x, ct, aug, x2)
    nblk = nsh // blk
    bps = slab // blk
    nslab = nsh // slab
    half = blk // 2            # 1024 = columns per psum group
    qcols = half               # q/psum free size
    outw = bps * half          # out-slab width (1024 per block)

    nc = bacc.Bacc("TRN2", target_bir_lowering=False, debug=False)
    # aug/selt are zero-padded to K=128: matmuls with tiny contraction (K=2)
    # keep the PE activity monitor from ever un-throttling the clock (HW-
    # measured: a 1/3 mix of K=2 MMs pins every MM at 1.2 GHz instead of 2.4).
    xt = nc.dram_tensor("xt", [D, nsh], xdt, kind="ExternalInput").ap()
    ctb = nc.dram_tensor("ctb", [128, 128], xdt, kind="ExternalInput").ap()
    aug = nc.dram_tensor("aug", [128, K], xdt, kind="ExternalInput").ap()
    x2d = nc.dram_tensor("x2d", [2, nsh], xdt, kind="ExternalInput").ap()
    sel = nc.dram_tensor("sel", [128, 32], f16, kind="ExternalInput").ap()
    selt = nc.dram_tensor("selt", [128, 128], f16, kind="ExternalInput").ap()
    selt2 = nc.dram_tensor("selt2", [128, 128], f16, kind="ExternalInput").ap()
    qo = nc.dram_tensor("q", [128, nsh // 2], f16, kind="ExternalOutput").ap()

    with tile.TileContext(nc) as tc, ExitStack() as ctx, \
            nc.allow_low_precision(reason="f16 q validated against reference"):
        wp = ctx.enter_context(tc.tile_pool(name="w", bufs=1))
        io = ctx.enter_context(tc.tile_pool(name="io", bufs=3))
        qp = ctx.enter_context(tc.tile_pool(name="qp", bufs=4))
        sv = ctx.enter_context(tc.tile_pool(name="sv", bufs=2))
        op = ctx.enter_context(tc.tile_pool(name="op", bufs=2))
        pp = ctx.enter_context(tc.tile_pool(name="ps", bufs=2, space="PSUM"))
        sp = ctx.enter_context(tc.tile_pool(name="sp", bufs=1, space="PSUM"))
        bp = ctx.enter_context(tc.tile_pool(name="bp", bufs=1, space="PSUM"))

        # one-time constants
        ctb_sb = wp.tile([128, 128], xdt, tag="ctb")
        nc.sync.dma_start(ctb_sb, ctb)
        aug_sb = wp.tile([128, K], xdt, tag="aug")
        nc.sync.dma_start(aug_sb, aug)
        sel_sb = wp.tile([128, 32], f16, tag="sel")
        nc.sync.dma_start(sel_sb, sel)
        selt_sb = wp.tile([128, 128], f16, tag="selt")
        nc.sync.dma_start(selt_sb, selt)
        selt2_sb = wp.tile([128, 128], f16, tag="selt2")
        nc.sync.dma_start(selt2_sb, selt2)

        # Manually double-buffered K=128 moving tiles whose rows 2:128 are
        # multiplied by stationary zeros: memset once so the garbage rows
        # can never be NaN (0 * NaN = NaN would poison the psum).
        x2ts = [wp.tile([128, slab], xdt, tag=f"x2t{j}", name=f"x2t{j}")
                for j in range(2)]
        nc.vector.memset(x2ts[0], 0.0)
        nc.gpsimd.memset(x2ts[1], 0.0)
        sis = [wp.tile([128, 512], f16, tag=f"si{j}", name=f"si{j}")
               for j in range(2)]
        for j in range(2):
            nc.vector.memset(sis[j], 0.0)

        xs = [None] * nslab    # (xt0s, xt1s) per live slab
        x2s = [None] * nslab   # x2 tile per live slab
        qs = [None] * nblk     # q tile per live block
        svs = [None] * nblk    # (sinv_h0, sinv_h1)
        outs = [None] * nslab  # out slab tile

        def load_slab(s):
            x0 = io.tile([128, slab], xdt, tag="xt0")
            x1 = io.tile([128, slab], xdt, tag="xt1")
            nc.sync.dma_start(x0, xt[0:128, s * slab:(s + 1) * slab])
            nc.sync.dma_start(x1, xt[128:256, s * slab:(s + 1) * slab])
            xs[s] = (x0, x1)

        def load_x2(s):
            # x2ts are manually double-buffered: this DMA must be emitted
            # only after every aug-MM read of slab s-2 (same buffer) is
            # already in the program, or the scheduler will order those
            # reads AFTER this write (reading slab s's x2 for slab s-2).
            x2t = x2ts[s % 2]
            nc.sync.dma_start(x2t[0:2, :], x2d[:, s * slab:(s + 1) * slab])
            x2s[s] = x2t

        def main_mms(b):
            s, bi = b // bps, b % bps
            x0, x1 = xs[s]
            x2t = x2s[s]
            ps = pp.tile([128, half], f32, tag="d2")
            for g in range(2):
                rows = slice(g * K, (g + 1) * K)
                tpos = (0, g * K) if g else None
                for h in range(2):
                    sl = slice(h * 512, (h + 1) * 512)
                    xoff = bi * blk + g * half + h * 512
                    xsl = slice(xoff, xoff + 512)
                    nc.tensor.matmul(ps[rows, sl], ctb_sb[:, 0:K], x0[:, xsl],
                                     start=True, stop=False, tile_position=tpos)
                    nc.tensor.matmul(ps[rows, sl], ctb_sb[:, K:2 * K], x1[:, xsl],
                                     start=False, stop=False, tile_position=tpos)
                    nc.tensor.matmul(ps[rows, sl], aug_sb, x2t[:, xsl],
                                     start=False, stop=True, tile_position=tpos)
            # q = 1/(1+d2) = exp(-ln(1+d2)) on the (otherwise idle) ACT engine
            u = qp.tile([128, qcols], f16, tag="u")
            nc.scalar.activation(u, ps, func=mybir.ActivationFunctionType.Ln,
                                 scale=1.0)
            q = qp.tile([128, qcols], f16, tag="q")
            nc.scalar.activation(q, u, func=mybir.ActivationFunctionType.Exp,
                                 scale=-1.0)
            qs[b] = q

        def norm_a(b):
            # both 512-halves' group sums packed into one [34, 512] psum:
            # h0 at rows 0:2, h1 at rows 32:34 (base-partition-32 aligned),
            # so one approx + one cast cover both.
            # sel's columns 2:32 are all-ones dummies so rows 2:32 of the
            # psum hold finite sums -- the packed reciprocal below must not
            # see stale/zero psum (1/0 -> inf -> 0*inf = NaN at the bcast).
            q = qs[b]
            st = sp.tile([64, 512], f32, tag="sh")
            nc.tensor.matmul(st[0:32, :], sel_sb, q[:, 0:512],
                             start=True, stop=True)
            nc.tensor.matmul(st[32:64, :], sel_sb, q[:, 512:1024],
                             start=True, stop=True, tile_position=(0, 32))
            sf = sv.tile([34, 512], f32, tag="sf")
            nc.vector.reciprocal_approx_fast(sf, st[0:34, :])
            si = sis[b % 2]
            nc.vector.tensor_copy(si[0:34, :], sf)
            svs[b] = si

        def norm_b(b):
            s, bi = b // bps, b % bps
            si = svs[b]
            bc = bp.tile([128, half], f32, tag="bc")
            nc.tensor.matmul(bc[:, 0:512], selt_sb, si, start=True, stop=True)
            nc.tensor.matmul(bc[:, 512:1024], selt2_sb, si,
                             start=True, stop=True)
            if bi == 0:
                ot = op.tile([128, outw], f16, tag="out")
                outs[s] = ot
            nc.vector.tensor_tensor(outs[s][:, bi * half:(bi + 1) * half],
                                    qs[b], bc, op=mybir.AluOpType.mult)
            if bi == bps - 1:
                nc.scalar.dma_start(
                    qo[:, s * outw:(s + 1) * outw], outs[s])

        # slab 0 loads in block-sized pieces so block 0's matmuls can start
        # after ~0.5 MB instead of 2 MB per half.
        x0f = io.tile([128, slab], xdt, tag="xt0")
        x1f = io.tile([128, slab], xdt, tag="xt1")
        for bi in range(bps):
            cs = slice(bi * blk, (bi + 1) * blk)
            nc.sync.dma_start(x0f[:, cs], xt[0:128, cs])
            nc.sync.dma_start(x1f[:, cs], xt[128:256, cs])
        xs[0] = (x0f, x1f)
        load_x2(0)
        if nslab > 1:
            load_slab(1)
            load_x2(1)
        if nslab > 2:
            load_slab(2)

        # PE warm-up burst: ~5us of dependency-free matmuls so HAM reaches
        # K=8/8 while the first input slab is still loading.
        wmw = wp.tile([128, K], xdt, tag="wmw")
        nc.vector.memset(wmw, 0.0)
        wmm = wp.tile([128, 512], xdt, tag="wmm")
        nc.vector.memset(wmm, 0.0)
        wps = pp.tile([128, half], f32, tag="d2")
        for j in range(24):
            nc.tensor.matmul(wps[0:K, 0:512], wmw, wmm,
                             start=True, stop=True)

        # norm stages for older blocks are emitted BEFORE this iteration's
        # main matmuls: their inputs are >=1 iteration old, so the tensor
        # queue never stalls waiting on the vector/scalar chain.
        for i in range(nblk + 3):
            if i < nblk and i % bps == 0 and i // bps + 3 < nslab:
                load_slab(i // bps + 3)
            if 2 <= i and i - 2 < nblk:
                norm_a(i - 2)
            if 3 <= i:
                norm_b(i - 3)
            if i < nblk:
                main_mms(i)
                if (i + 1) % bps == 0 and i // bps + 2 < nslab:
                    load_x2(i // bps + 2)

    nc.compile()
    return nc


_CACHE = {}


def _get_nc():
    if "nc" not in _CACHE:
        _CACHE["nc"] = _build()
    return _CACHE["nc"]


def _prep_inputs(x, c):
    """Build per-core input maps (host-side shard + layout prep)."""
    import ml_dtypes

    f16 = np.float16
    xdt = ml_dtypes.bfloat16
    x = np.asarray(x, dtype=np.float32)
    c = np.asarray(c, dtype=np.float32)
    assert x.shape == (N, D) and c.shape == (K, D)

    c2p1 = 1.0 + np.sum(c * c, axis=1)                     # (K,)
    aug = np.zeros((128, K), np.float32)                   # K=128 zero-padded
    aug[0] = c2p1
    aug[1] = 1.0
    aug = aug.astype(xdt)
    ctb = np.ascontiguousarray(
        np.concatenate([(-2.0 * c).T[0:128, :], (-2.0 * c).T[128:256, :]],
                       axis=1)).astype(xdt)                # [128, 128]
    sel = np.ones((128, 32), f16)      # cols 2:32 = dummy all-ones columns
    sel[:, 0] = 0.0
    sel[:, 1] = 0.0
    sel[0:K, 0] = 1.0
    sel[K:128, 1] = 1.0
    selt = np.zeros((128, 128), f16)                       # K=128 zero-padded
    selt[0, 0:K] = 1.0
    selt[1, K:128] = 1.0
    selt2 = np.zeros((128, 128), f16)                      # h1 variant (rows 32/33)
    selt2[32, 0:K] = 1.0
    selt2[33, K:128] = 1.0

    x2 = np.sum(x.astype(np.float32) ** 2, axis=1)         # (N,) f32

    in_maps = []
    for i in range(NCORES):
        xsh = x[i * NSH:(i + 1) * NSH]
        xts = np.ascontiguousarray(xsh.T.astype(xdt))      # [D, NSH]
        x2d = np.empty((2, NSH), xdt)
        x2d[0] = 1.0
        x2d[1] = x2[i * NSH:(i + 1) * NSH].astype(xdt)
        in_maps.append({"xt": xts, "ctb": ctb, "aug": aug, "x2d": x2d,
                        "sel": sel, "selt": selt, "selt2": selt2})
    return in_maps


def _postprocess(results):
    """[128, NSH/2] f16 per core -> full [N, K] f32."""
    outs = []
    for r in results:
        qt = np.asarray(r["q"]).astype(np.float32)          # [128, 32768]
        a = qt.reshape(2, K, NSLAB, BPS, BLK // 2)          # [g, k, s, bi, j]
        outs.append(a.transpose(2, 3, 0, 4, 1).reshape(NSH, K))
    return np.concatenate(outs, axis=0)


def kernel(inputs, clusters):
    from concourse.bass_utils import run_bass_kernel_spmd

    nc = _get_nc()
    in_maps = _prep_inputs(inputs, clusters)
    res = run_bass_kernel_spmd(nc, in_maps, core_ids=list(range(NCORES)))
    return _postprocess(res.results)


# revision 46
# speedup vs baseline: 1.0268x; 1.0133x over previous
"""Trainium2 Bass kernel for nn_ClusteringLayer (vq_codebook, Student-t assignments).

Computes, for x in R^{N x D} and clusters c in R^{K x D}:
    d2[n,k] = ||x_n - c_k||^2
    q = 1 / (1 + d2)            (Student-t, alpha=1, power=(alpha+1)/2=1)
    out = q / q.sum(-1, keepdims=True)

Strategy (data-parallel over 8 NeuronCores, cluster table replicated):
  - host: shard x along N (65536 rows/core), transpose+cast each shard to
    f16 X^T [D, Nsh]; precompute x2[n] = ||x_n||^2 (f16) and the cluster
    constants (-2C)^T (f16 stationary) and 1+||c_k||^2.
  - device, per 2048-column block, with psum packed [2 groups x 64 k]:
      psum[kp, j] = sum_d (-2c)[d,k] x[d,n]  +  (1 + c2[k]) + x2[n]
                    (2 main matmuls + 1 tiny K=2 "aug" matmul per 512-slice;
                     the aug matmul streams [ones; x2] against [c2p1; ones])
      q  = 1/psum                    (DVE reciprocal, f16)          [n-major: no!]
      s  = indicator-matmul over the k-partitions -> [2, 512] psums
      sinv = 1/s                     (DVE)
      bcast = selT-matmul            (replicates sinv to all 64 k rows)
      out = q * bcast                (DVE)  -> accumulated in slab tile
  - slab-grained (8192 cols) input/output DMAs for large descriptors.
  - host: upcast f16 -> f32, unpack [k, n] blocks to [n, k], concat shards.
"""

import numpy as np
from contextlib import ExitStack


def _patch_act_tables():
    """Make Ln and Exp resolve to the single set that contains both
    (natural_log_exp_and_others), so the kernel pays one ACT_TABLE_LOAD
    instead of alternating sets every block.  Only values are modified --
    set order (and hence act_func_set_id indices) is preserved."""
    import functools
    from concourse import hw_specs, bacc, mybir

    if getattr(hw_specs, "_act_tables_patched", False):
        return
    orig = hw_specs.get_activation_tables

    @functools.cache
    def patched(arch):
        t = dict(orig(arch))
        ln = mybir.ActivationFunctionType.Ln
        ex = mybir.ActivationFunctionType.Exp
        out = {}
        for name, funcs in t.items():
            if name != "natural_log_exp_and_others" and (ln in funcs or ex in funcs):
                funcs = funcs - {ln, ex}
            out[name] = funcs
        return out

    hw_specs.get_activation_tables = patched
    bacc.get_activation_tables = patched
    hw_specs._act_tables_patched = True

N, D, K = 524288, 256, 64
NCORES = 8
NSH = N // NCORES      # 65536 rows per core
BLK = 2048             # n-columns per psum block (2 groups x 1024)
SLAB = 8192            # n-columns per DMA slab (4 blocks)
NBLK = NSH // BLK      # 32
NSLAB = NSH // SLAB    # 8
BPS = SLAB // BLK      # blocks per slab = 4


def _build(nsh=NSH, blk=BLK, slab=SLAB):
    import concourse.bacc as bacc
    import concourse.tile as tile
    from concourse import mybir

    _patch_act_tables()

    f32 = mybir.dt.float32
    f16 = mybir.dt.float16
    xdt = mybir.dt.bfloat16   # dtype of the main GEMM path (x, ct, aug, x2)
    nblk = nsh // blk
    bps = slab // blk
    nslab = nsh // slab
    half = blk // 2            # 1024 = columns per psum group
    qcols = half               # q/psum free size
    outw = bps * half          # out-slab width (1024 per block)

    nc = bacc.Bacc("TRN2", target_bir_lowering=False, debug=False)
    # aug/selt are zero-padded to K=128: matmuls with tiny contraction (K=2)
    # keep the PE activity monitor from ever un-throttling the clock (HW-
    # measured: a 1/3 mix of K=2 MMs pins every MM at 1.2 GHz instead of 2.4).
    xt = nc.dram_tensor("xt", [D, nsh], xdt, kind="ExternalInput").ap()
    ctb = nc.dram_tensor("ctb", [128, 128], xdt, kind="ExternalInput").ap()
    aug = nc.dram_tensor("aug", [128, K], xdt, kind="ExternalInput").ap()
    x2d = nc.dram_tensor("x2d", [2, nsh], xdt, kind="ExternalInput").ap()
    sel = nc.dram_tensor("sel", [128, 32], f16, kind="ExternalInput").ap()
    selt = nc.dram_tensor("selt", [128, 128], f16, kind="ExternalInput").ap()
    selt2 = nc.dram_tensor("selt2", [128, 128], f16, kind="ExternalInput").ap()
    qo = nc.dram_tensor("q", [128, nsh // 2], f16, kind="ExternalOutput").ap()

    with tile.TileContext(nc) as tc, ExitStack() as ctx, \
            nc.allow_low_precision(reason="f16 q validated against reference"):
        wp = ctx.enter_context(tc.tile_pool(name="w", bufs=1))
        io = ctx.enter_context(tc.tile_pool(name="io", bufs=3))
        qp = ctx.enter_context(tc.tile_pool(name="qp", bufs=4))
        sv = ctx.enter_context(tc.tile_pool(name="sv", bufs=2))
        op = ctx.enter_context(tc.tile_pool(name="op", bufs=2))
        pp = ctx.enter_context(tc.tile_pool(name="ps", bufs=2, space="PSUM"))
        sp = ctx.enter_context(tc.tile_pool(name="sp", bufs=1, space="PSUM"))
        bp = ctx.enter_context(tc.tile_pool(name="bp", bufs=1, space="PSUM"))

        # one-time constants
        ctb_sb = wp.tile([128, 128], xdt, tag="ctb")
        nc.sync.dma_start(ctb_sb, ctb)
        aug_sb = wp.tile([128, K], xdt, tag="aug")
        nc.sync.dma_start(aug_sb, aug)
        sel_sb = wp.tile([128, 32], f16, tag="sel")
        nc.sync.dma_start(sel_sb, sel)
        selt_sb = wp.tile([128, 128], f16, tag="selt")
        nc.sync.dma_start(selt_sb, selt)
        selt2_sb = wp.tile([128, 128], f16, tag="selt2")
        nc.sync.dma_start(selt2_sb, selt2)

        # Manually double-buffered K=128 moving tiles whose rows 2:128 are
        # multiplied by stationary zeros: memset once so the garbage rows
        # can never be NaN (0 * NaN = NaN would poison the psum).
        x2ts = [wp.tile([128, slab], xdt, tag=f"x2t{j}", name=f"x2t{j}")
                for j in range(2)]
        nc.vector.memset(x2ts[0], 0.0)
        nc.gpsimd.memset(x2ts[1], 0.0)
        sis = [wp.tile([128, 512], f16, tag=f"si{j}", name=f"si{j}")
               for j in range(2)]
        for j in range(2):
            nc.vector.memset(sis[j], 0.0)

        xs = [None] * nslab    # (xt0s, xt1s) per live slab
        x2s = [None] * nslab   # x2 tile per live slab
        qs = [None] * nblk     # q tile per live block
        svs = [None] * nblk    # (sinv_h0, sinv_h1)
        outs = [None] * nslab  # out slab tile

        def load_slab(s):
            x0 = io.tile([128, slab], xdt, tag="xt0")
            x1 = io.tile([128, slab], xdt, tag="xt1")
            nc.sync.dma_start(x0, xt[0:128, s * slab:(s + 1) * slab])
            nc.sync.dma_start(x1, xt[128:256, s * slab:(s + 1) * slab])
            xs[s] = (x0, x1)

        def load_x2(s):
            # x2ts are manually double-buffered: this DMA must be emitted
            # only after every aug-MM read of slab s-2 (same buffer) is
            # already in the program, or the scheduler will order those
            # reads AFTER this write (reading slab s's x2 for slab s-2).
            x2t = x2ts[s % 2]
            nc.sync.dma_start(x2t[0:2, :], x2d[:, s * slab:(s + 1) * slab])
            x2s[s] = x2t

        def main_mms(b):
            s, bi = b // bps, b % bps
            x0, x1 = xs[s]
            x2t = x2s[s]
            ps = pp.tile([128, half], f32, tag="d2")
            for g in range(2):
                rows = slice(g * K, (g + 1) * K)
                tpos = (0, g * K) if g else None
                for h in range(2):
                    sl = slice(h * 512, (h + 1) * 512)
                    xoff = bi * blk + g * half + h * 512
                    xsl = slice(xoff, xoff + 512)
                    nc.tensor.matmul(ps[rows, sl], ctb_sb[:, 0:K], x0[:, xsl],
                                     start=True, stop=False, tile_position=tpos)
                    nc.tensor.matmul(ps[rows, sl], ctb_sb[:, K:2 * K], x1[:, xsl],
                                     start=False, stop=False, tile_position=tpos)
                    nc.tensor.matmul(ps[rows, sl], aug_sb, x2t[:, xsl],
                                     start=False, stop=True, tile_position=tpos)
            # q = 1/(1+d2) = exp(-ln(1+d2)) on the (otherwise idle) ACT engine
            u = qp.tile([128, qcols], f16, tag="u")
            nc.scalar.activation(u, ps, func=mybir.ActivationFunctionType.Ln,
                                 scale=1.0)
            q = qp.tile([128, qcols], f16, tag="q")
            nc.scalar.activation(q, u, func=mybir.ActivationFunctionType.Exp,
                                 scale=-1.0)
            qs[b] = q

        def norm_a(b):
            # both 512-halves' group sums packed into one [34, 512] psum:
            # h0 at rows 0:2, h1 at rows 32:34 (base-partition-32 aligned),
            # so one approx + one cast cover both.
            # sel's columns 2:32 are all-ones dummies so rows 2:32 of the
            # psum hold finite sums -- the packed reciprocal below must not
            # see stale/zero psum (1/0 -> inf -> 0*inf = NaN at the bcast).
            q = qs[b]
            st = sp.tile([64, 512], f32, tag="sh")
            nc.tensor.matmul(st[0:32, :], sel_sb, q[:, 0:512],
                             start=True, stop=True)
            nc.tensor.matmul(st[32:64, :], sel_sb, q[:, 512:1024],
                             start=True, stop=True, tile_position=(0, 32))
            sf = sv.tile([34, 512], f32, tag="sf")
            nc.vector.reciprocal_approx_fast(sf, st[0:34, :])
            si = sis[b % 2]
            nc.vector.tensor_copy(si[0:34, :], sf)
            svs[b] = si

        def norm_b(b):
            s, bi = b // bps, b % bps
            si = svs[b]
            bc = bp.tile([128, half], f32, tag="bc")
            nc.tensor.matmul(bc[:, 0:512], selt_sb, si, start=True, stop=True)
            nc.tensor.matmul(bc[:, 512:1024], selt2_sb, si,
                             start=True, stop=True)
            if bi == 0:
                ot = op.tile([128, outw], f16, tag="out")
                outs[s] = ot
            nc.vector.tensor_tensor(outs[s][:, bi * half:(bi + 1) * half],
                                    qs[b], bc, op=mybir.AluOpType.mult)
            if bi == bps - 1:
                nc.scalar.dma_start(
                    qo[:, s * outw:(s + 1) * outw], outs[s])

        # slab 0 loads in block-sized pieces so block 0's matmuls can start
        # after ~0.5 MB instead of 2 MB per half.
        x0f = io.tile([128, slab], xdt, tag="xt0")
        x1f = io.tile([128, slab], xdt, tag="xt1")
        for bi in range(bps):
            cs = slice(bi * blk, (bi + 1) * blk)
            nc.sync.dma_start(x0f[:, cs], xt[0:128, cs])
            nc.sync.dma_start(x1f[:, cs], xt[128:256, cs])
        xs[0] = (x0f, x1f)
        load_x2(0)
        if nslab > 1:
            load_slab(1)
            load_x2(1)
        if nslab > 2:
            load_slab(2)

        # norm stages for older blocks are emitted BEFORE this iteration's
        # main matmuls: their inputs are >=1 iteration old, so the tensor
        # queue never stalls waiting on the vector/scalar chain.
        for i in range(nblk + 3):
            if i < nblk and i % bps == 0 and i // bps + 3 < nslab:
                load_slab(i // bps + 3)
            if 2 <= i and i - 2 < nblk:
                norm_a(i - 2)
            if 3 <= i:
                norm_b(i - 3)
            if i < nblk:
                main_mms(i)
                if (i + 1) % bps == 0 and i // bps + 2 < nslab:
                    load_x2(i // bps + 2)

    nc.compile()
    return nc


_CACHE = {}


def _get_nc():
    if "nc" not in _CACHE:
        _CACHE["nc"] = _build()
    return _CACHE["nc"]


def _prep_inputs(x, c):
    """Build per-core input maps (host-side shard + layout prep)."""
    import ml_dtypes

    f16 = np.float16
    xdt = ml_dtypes.bfloat16
    x = np.asarray(x, dtype=np.float32)
    c = np.asarray(c, dtype=np.float32)
    assert x.shape == (N, D) and c.shape == (K, D)

    c2p1 = 1.0 + np.sum(c * c, axis=1)                     # (K,)
    aug = np.zeros((128, K), np.float32)                   # K=128 zero-padded
    aug[0] = c2p1
    aug[1] = 1.0
    aug = aug.astype(xdt)
    ctb = np.ascontiguousarray(
        np.concatenate([(-2.0 * c).T[0:128, :], (-2.0 * c).T[128:256, :]],
                       axis=1)).astype(xdt)                # [128, 128]
    sel = np.ones((128, 32), f16)      # cols 2:32 = dummy all-ones columns
    sel[:, 0] = 0.0
    sel[:, 1] = 0.0
    sel[0:K, 0] = 1.0
    sel[K:128, 1] = 1.0
    selt = np.zeros((128, 128), f16)                       # K=128 zero-padded
    selt[0, 0:K] = 1.0
    selt[1, K:128] = 1.0
    selt2 = np.zeros((128, 128), f16)                      # h1 variant (rows 32/33)
    selt2[32, 0:K] = 1.0
    selt2[33, K:128] = 1.0

    x2 = np.sum(x.astype(np.float32) ** 2, axis=1)         # (N,) f32

    in_maps = []
    for i in range(NCORES):
        xsh = x[i * NSH:(i + 1) * NSH]
        xts = np.ascontiguousarray(xsh.T.astype(xdt))      # [D, NSH]
        x2d = np.empty((2, NSH), xdt)
        x2d[0] = 1.0
        x2d[1] = x2[i * NSH:(i + 1) * NSH].astype(xdt)
        in_maps.append({"xt": xts, "ctb": ctb, "aug": aug, "x2d": x2d,
                        "sel": sel, "selt": selt, "selt2": selt2})
    return in_maps


def _postprocess(results):
    """[128, NSH/2] f16 per core -> full [N, K] f32."""
    outs = []
    for r in results:
        qt = np.asarray(r["q"]).astype(np.float32)          # [128, 32768]
        a = qt.reshape(2, K, NSLAB, BPS, BLK // 2)          # [g, k, s, bi, j]
        outs.append(a.transpose(2, 3, 0, 4, 1).reshape(NSH, K))
    return np.concatenate(outs, axis=0)


def kernel(inputs, clusters):
    from concourse.bass_utils import run_bass_kernel_spmd

    nc = _get_nc()
    in_maps = _prep_inputs(inputs, clusters)
    res = run_bass_kernel_spmd(nc, in_maps, core_ids=list(range(NCORES)))
    return _postprocess(res.results)
